# revision 18
# baseline (speedup 1.0000x reference)
"""HEPOS BART cross-attention Trainium2 kernel.

Shapes (hardcoded): B=2, Tq=1024, Tk=8192, E=1024, H=16, D=64, stride=16,
m = Tk//stride = 512 keys per head.

Sharding: 8 cores = 2 batches x 4 head-groups (4 heads each).
Each core computes, for its batch b and heads hg=[4g..4g+3]:
  QT   = (Wq_hg @ hs_b^T) * scale + bq  -> [256, 1024]   (d-major)
  KgT  = Wk_h @ kvg_h^T                 -> [64, 512] per head
  VgT  = Wv_h @ kvg_h^T -> PE-transpose -> Vg' [512, 65] (ones col -> rowsum)
  ST   = Kg @ Q^T (scoresT)             -> [512, 1024] per head
  ET   = exp(ST)                        (no max-subtraction; scores are O(1))
  OT'  = Vg'^T @ ET                     -> [65, 1024]: rows 0-63 out, row 64 sum
  OT   = OT'[0:64] * (1/OT'[64])        -> outT_all [256, 1024]
  partial = outT_all^T @ WoT_c          -> [1024, 1024]  (row-parallel)
Host sums the 4 partials per batch and adds (bv @ Wo.T + bo).
bk is dropped: a constant shift of every gathered key adds the same value to
every score in a softmax row, which cancels exactly.

All matmuls run as float32r (fp32 data, 1 cycle/row at N=512).
"""

import numpy as np

import concourse.bass as bass
import concourse.bacc as bacc
import concourse.tile as tile
from concourse import library_config, mybir
from concourse.masks import make_identity

B, Tq, Tk, E, H, D = 2, 1024, 8192, 1024, 16, 64
STRIDE = 16
M = Tk // STRIDE          # 512 keys per head
HPC = 4                   # heads per core
NCORES = 8
F32 = mybir.dt.float32
F32R = mybir.dt.float32r


def r(ap):
    """View an SBUF/PSUM AP as float32r for the tensor engine."""
    return ap.bitcast(F32R)


def build_program():
    nc = bacc.Bacc("TRN2", target_bir_lowering=False)

    hsT = nc.dram_tensor("hsT", [8, 128, Tq], F32R, kind="ExternalInput")
    kvgT = nc.dram_tensor("kvgT", [HPC, 8, 128, M], F32R, kind="ExternalInput")
    wqT = nc.dram_tensor("wqT", [8, 128, 256], F32R, kind="ExternalInput")
    bqh = nc.dram_tensor("bqh", [HPC, D, 1], F32, kind="ExternalInput")
    wkT = nc.dram_tensor("wkT", [HPC, 8, 128, D], F32R, kind="ExternalInput")
    wvT = nc.dram_tensor("wvT", [HPC, 8, 128, D], F32R, kind="ExternalInput")
    woT = nc.dram_tensor("woT", [2, 128, E], F32R, kind="ExternalInput")
    out = nc.dram_tensor("out", [8, 128, E], F32, kind="ExternalOutput")

    with tile.TileContext(nc) as tc:
        with (
            tc.tile_pool(name="consts", bufs=1) as consts,
            tc.tile_pool(name="kvpool", bufs=2) as kvpool,
            tc.tile_pool(name="exppool", bufs=2) as exppool,
            tc.tile_pool(name="kgpool", bufs=1) as kgpool,
            tc.tile_pool(name="vgpool", bufs=2) as vgpool,
            tc.tile_pool(name="rpool", bufs=4) as rpool,
            tc.tile_pool(name="opool", bufs=3) as opool,
            tc.tile_pool(name="ps_a", bufs=2, space="PSUM") as ps_a,
            tc.tile_pool(name="ps_s", bufs=3, space="PSUM") as ps_s,
            tc.tile_pool(name="ps_o", bufs=3, space="PSUM") as ps_o,
        ):
            # ---- persistent SBUF tiles -------------------------------------
            hsT_sb = consts.tile([128, 8 * Tq], F32R)
            wq_sb = consts.tile([128, 8 * 256], F32R)
            wk_sb = consts.tile([128, HPC * 8 * D], F32R)
            wv_sb = consts.tile([128, HPC * 8 * D], F32R)
            wo_sb = consts.tile([128, 2 * E], F32R)
            ident = consts.tile([128, 128], F32)
            qt_sb = [consts.tile([D, Tq], F32R, name=f"qt{h}") for h in range(HPC)]
            outT_sb = [consts.tile([128, Tq], F32R, name=f"outT{dd}") for dd in range(2)]

            make_identity(nc, ident)
            ones_f = consts.tile([1, 512], F32)
            nc.vector.memset(ones_f[:], 1.0)
            ones_sb = consts.tile([1, 512], F32R)
            nc.vector.tensor_copy(ones_sb[:], ones_f[:])
            onescol_f = consts.tile([128, HPC, 1], F32)
            nc.vector.memset(onescol_f[:], 1.0)

            # ---- input DMAs ------------------------------------------------
            for e in range(8):
                nc.sync.dma_start(out=wq_sb[:, e * 256:(e + 1) * 256], in_=wqT[e])
            for e in range(8):
                nc.sync.dma_start(out=hsT_sb[:, e * Tq:(e + 1) * Tq], in_=hsT[e])
            for h in range(HPC):
                for e in range(8):
                    nc.sync.dma_start(
                        out=wk_sb[:, (h * 8 + e) * D:(h * 8 + e + 1) * D],
                        in_=wkT[h, e])
                    nc.sync.dma_start(
                        out=wv_sb[:, (h * 8 + e) * D:(h * 8 + e + 1) * D],
                        in_=wvT[h, e])
            for dd in range(2):
                nc.sync.dma_start(out=wo_sb[:, dd * E:(dd + 1) * E], in_=woT[dd])

            bq_tiles = [consts.tile([D, 1], F32, name=f"bq{h}") for h in range(HPC)]
            for h in range(HPC):
                nc.sync.dma_start(out=bq_tiles[h][:], in_=bqh[h])

            # ---- phase 1: QT projection ------------------------------------
            # psum [128, 512] holds a head pair (rows 0-63 head 2p, 64-127 head 2p+1)
            for pair in range(2):
                for tqt in range(2):
                    ps_qt = ps_s.tile([128, 512], F32, tag="ps_s")
                    for e in range(8):
                        nc.tensor.matmul(
                            ps_qt[:],
                            r(wq_sb[:, e * 256 + pair * 128: e * 256 + (pair + 1) * 128]),
                            r(hsT_sb[:, e * Tq + tqt * 512: e * Tq + tqt * 512 + 512]),
                            start=(e == 0), stop=(e == 7))
                    for sub in range(2):
                        h = 2 * pair + sub
                        nc.scalar.activation(
                            qt_sb[h][:, tqt * 512: tqt * 512 + 512],
                            ps_qt[sub * 64:(sub + 1) * 64, :],
                            mybir.ActivationFunctionType.Identity,
                            bias=bq_tiles[h][:])

            # ---- phase 2: per-head K/V proj + attention --------------------
            for h in range(HPC):
                kvg_sb = kvpool.tile([128, 8 * M], F32R, tag="kvg")
                for e in range(8):
                    nc.sync.dma_start(
                        out=kvg_sb[:, e * M:(e + 1) * M], in_=kvgT[h, e])

                # K^T_g [64, 512]
                kg_sb = kgpool.tile([D, M], F32R, tag="kg", bufs=2)
                ps_kg = ps_a.tile([D, M], F32, tag="ps_a")
                for e in range(8):
                    nc.tensor.matmul(
                        ps_kg[:],
                        r(wk_sb[:, (h * 8 + e) * D:(h * 8 + e + 1) * D]),
                        r(kvg_sb[:, e * M:(e + 1) * M]),
                        start=(e == 0), stop=(e == 7))
                nc.vector.tensor_copy(kg_sb[:], ps_kg[:])

                # V^T_g [64, 512] -> transpose into Vg' [4][128, 65]
                vgT_sb = vgpool.tile([D, M], F32, tag="vgT")
                ps_vg = ps_a.tile([D, M], F32, tag="ps_a")
                for e in range(8):
                    nc.tensor.matmul(
                        ps_vg[:],
                        r(wv_sb[:, (h * 8 + e) * D:(h * 8 + e + 1) * D]),
                        r(kvg_sb[:, e * M:(e + 1) * M]),
                        start=(e == 0), stop=(e == 7))
                nc.vector.tensor_copy(vgT_sb[:], ps_vg[:])

                vgp_sb = vgpool.tile([128, 4, D + 1], F32R, tag="vgp")
                nc.vector.tensor_copy(vgp_sb[:, :, D:D + 1], onescol_f[:])
                for mc in range(4):
                    ps_vt = ps_a.tile([128, D], F32, tag="ps_a")
                    nc.tensor.transpose(
                        ps_vt[:], vgT_sb[:, mc * 128:(mc + 1) * 128],
                        ident[0:D, 0:D])
                    nc.vector.tensor_copy(vgp_sb[:, mc, 0:D], ps_vt[:])

                # scoresT -> exp -> attnT @ Vg' -> normalize
                expT_sb = exppool.tile([128, 4, Tq], F32R, tag="expT")
                for tqt in range(2):
                    for mc in range(4):
                        ps_sc = ps_s.tile([128, 512], F32, tag="ps_s")
                        nc.tensor.matmul(
                            ps_sc[:],
                            r(kg_sb[:, mc * 128:(mc + 1) * 128]),
                            r(qt_sb[h][:, tqt * 512: tqt * 512 + 512]),
                            start=True, stop=True)
                        nc.scalar.activation(
                            expT_sb[:, mc, tqt * 512: tqt * 512 + 512],
                            ps_sc[:],
                            mybir.ActivationFunctionType.Exp)

                    ps_ov = ps_o.tile([128, 512], F32, tag="ps_o")
                    for mc in range(4):
                        nc.tensor.matmul(
                            ps_ov[0:D + 1, :],
                            r(vgp_sb[:, mc, :]),
                            r(expT_sb[:, mc, tqt * 512: tqt * 512 + 512]),
                            start=(mc == 0), stop=(mc == 3))
                    rinv = rpool.tile([1, 512], F32R, tag="rinv")
                    with nc.allow_low_precision(reason="fp32r feed for PE bcast"):
                        nc.vector.reciprocal(rinv[:], ps_ov[D:D + 1, :])
                    ps_rb = ps_a.tile([D, 512], F32, tag="ps_a")
                    nc.tensor.matmul(ps_rb[:], ones_sb[:, 0:D], rinv[:],
                                     start=True, stop=True)
                    rinv_b = rpool.tile([D, 512], F32, tag="rinv_b")
                    nc.vector.tensor_copy(rinv_b[:], ps_rb[:])
                    nc.vector.tensor_tensor(
                        outT_sb[h // 2][(h % 2) * D:(h % 2 + 1) * D,
                                        tqt * 512: tqt * 512 + 512],
                        ps_ov[0:D, :],
                        rinv_b[:],
                        op=mybir.AluOpType.mult)

            # ---- phase 3: out-projection (row-parallel partial) ------------
            for t8 in range(8):
                for eot in range(2):
                    ps_po = ps_o.tile([128, 512], F32, tag="ps_o")
                    for dd in range(2):
                        nc.tensor.matmul(
                            ps_po[:],
                            r(outT_sb[dd][:, t8 * 128:(t8 + 1) * 128]),
                            r(wo_sb[:, dd * E + eot * 512: dd * E + eot * 512 + 512]),
                            start=(dd == 0), stop=(dd == 1))
                    po_sb = opool.tile([128, 512], F32, tag="po")
                    nc.vector.tensor_copy(po_sb[:], ps_po[:])
                    nc.sync.dma_start(
                        out=out[t8][:, eot * 512: eot * 512 + 512], in_=po_sb[:])

    nc.compile()
    return nc


_NC = None


def _get_nc():
    global _NC
    if _NC is None:
        _NC = build_program()
    return _NC


def round_fp32r(a):
    """Round-to-nearest-even to fp32r: 11-bit mantissa, low 12 bits zero."""
    u = np.ascontiguousarray(a, np.float32).view(np.uint32)
    u = u + 0x7FF + ((u >> 12) & 1)
    u &= np.uint32(0xFFFFF000)
    return u.view(np.float32)


def shard_inputs(hidden_states, key_value_states, Wq, bq, Wk, bk, Wv, bv, Wo, bo,
                 stride):
    stride = int(stride)
    assert stride == STRIDE
    scale = float(D) ** -0.5
    in_maps = []
    for c in range(NCORES):
        b, g = divmod(c, 4)
        h0 = g * HPC  # first global head of this core
        r0, r1 = h0 * D, (h0 + HPC) * D
        hsT_c = np.ascontiguousarray(hidden_states[b].T).reshape(8, 128, Tq)
        kvgT_c = np.empty((HPC, 8, 128, M), np.float32)
        for hl in range(HPC):
            rows = key_value_states[b, (h0 + hl)::STRIDE, :]  # [M, E]
            kvgT_c[hl] = np.ascontiguousarray(rows.T).reshape(8, 128, M)
        wqT_c = np.ascontiguousarray((Wq[r0:r1, :] * scale).T).reshape(8, 128, 256)
        bq_c = (bq[r0:r1] * scale).astype(np.float32).reshape(HPC, D, 1)
        wkT_c = np.ascontiguousarray(
            Wk[r0:r1, :].reshape(HPC, D, E).transpose(0, 2, 1)).reshape(
                HPC, 8, 128, D)
        wvT_c = np.ascontiguousarray(
            Wv[r0:r1, :].reshape(HPC, D, E).transpose(0, 2, 1)).reshape(
                HPC, 8, 128, D)
        woT_c = np.ascontiguousarray(Wo[:, r0:r1].T).reshape(2, 128, E)
        in_maps.append({
            "hsT": round_fp32r(hsT_c),
            "kvgT": round_fp32r(kvgT_c),
            "wqT": round_fp32r(wqT_c),
            "bqh": bq_c,
            "wkT": round_fp32r(wkT_c),
            "wvT": round_fp32r(wvT_c),
            "woT": round_fp32r(woT_c),
        })
    return in_maps


def combine_outputs(results, Wv, bv, Wo, bo):
    final_bias = (bv @ Wo.T + bo).astype(np.float32)  # [E]
    out = np.zeros((B, Tq, E), np.float32)
    for c in range(NCORES):
        b = c // 4
        out[b] += results[c]["out"].reshape(Tq, E)
    out += final_bias[None, None, :]
    return out


def kernel(hidden_states, key_value_states, Wq, bq, Wk, bk, Wv, bv, Wo, bo,
           stride, _trace=False, _trace_kwargs=None):
    from concourse.bass_utils import run_bass_kernel_spmd

    args = [np.asarray(x, np.float32) for x in
            (hidden_states, key_value_states, Wq, bq, Wk, bk, Wv, bv, Wo, bo)]
    (hidden_states, key_value_states, Wq, bq, Wk, bk, Wv, bv, Wo, bo) = args
    in_maps = shard_inputs(hidden_states, key_value_states, Wq, bq, Wk, bk,
                           Wv, bv, Wo, bo, stride)
    nc = _get_nc()
    res = run_bass_kernel_spmd(
        nc, in_maps, list(range(NCORES)),
        trace=_trace, **(_trace_kwargs or {}))
    out = combine_outputs(res.results, Wv, bv, Wo, bo)
    kernel.last_run = res
    return out


# revision 20
# speedup vs baseline: 1.0950x; 1.0950x over previous
"""HEPOS BART cross-attention Trainium2 kernel.

Shapes (hardcoded): B=2, Tq=1024, Tk=8192, E=1024, H=16, D=64, stride=16,
m = Tk//stride = 512 keys per head.

Sharding: 8 cores = 2 batches x 4 head-groups (4 heads each).
Each core computes, for its batch b and heads hg=[4g..4g+3]:
  QT   = (Wq_hg @ hs_b^T) * scale + bq  -> [256, 1024]   (d-major)
  KgT  = Wk_h @ kvg_h^T                 -> [64, 512] per head
  VgT  = Wv_h @ kvg_h^T -> PE-transpose -> Vg' [512, 65] (ones col -> rowsum)
  ST   = Kg @ Q^T (scoresT)             -> [512, 1024] per head
  ET   = exp(ST)                        (no max-subtraction; scores are O(1))
  OT'  = Vg'^T @ ET                     -> [65, 1024]: rows 0-63 out, row 64 sum
  OT   = OT'[0:64] * (1/OT'[64])        -> outT_all [256, 1024]
  partial = outT_all^T @ WoT_c          -> [1024, 1024]  (row-parallel)
Host sums the 4 partials per batch and adds (bv @ Wo.T + bo).
bk is dropped: a constant shift of every gathered key adds the same value to
every score in a softmax row, which cancels exactly.

All matmuls run as float32r (fp32 data, 1 cycle/row at N=512).
"""

import numpy as np

import concourse.bass as bass
import concourse.bacc as bacc
import concourse.tile as tile
from concourse import library_config, mybir
from concourse.masks import make_identity

B, Tq, Tk, E, H, D = 2, 1024, 8192, 1024, 16, 64
STRIDE = 16
M = Tk // STRIDE          # 512 keys per head
HPC = 4                   # heads per core
NCORES = 8
F32 = mybir.dt.float32
F32R = mybir.dt.float32r


def r(ap):
    """View an SBUF/PSUM AP as float32r for the tensor engine."""
    return ap.bitcast(F32R)


def build_program():
    nc = bacc.Bacc("TRN2", target_bir_lowering=False)

    hsT = nc.dram_tensor("hsT", [8, 128, Tq], F32R, kind="ExternalInput")
    kvgT = nc.dram_tensor("kvgT", [HPC, 8, 128, M], F32R, kind="ExternalInput")
    wqT = nc.dram_tensor("wqT", [8, 128, 256], F32R, kind="ExternalInput")
    bqh = nc.dram_tensor("bqh", [HPC, D, 1], F32, kind="ExternalInput")
    wkT = nc.dram_tensor("wkT", [HPC, 8, 128, D], F32R, kind="ExternalInput")
    wvT = nc.dram_tensor("wvT", [HPC, 8, 128, D], F32R, kind="ExternalInput")
    woT = nc.dram_tensor("woT", [2, 128, E], F32R, kind="ExternalInput")
    out = nc.dram_tensor("out", [8, 128, E], F32, kind="ExternalOutput")

    _dma_engs = [nc.sync, nc.scalar, nc.gpsimd]
    _dma_i = [0]

    def dma(out, in_):
        eng = _dma_engs[_dma_i[0] % len(_dma_engs)]
        _dma_i[0] += 1
        eng.dma_start(out=out, in_=in_)

    with tile.TileContext(nc) as tc:
        with (
            tc.tile_pool(name="consts", bufs=1) as consts,
            tc.tile_pool(name="kvpool", bufs=2) as kvpool,
            tc.tile_pool(name="exppool", bufs=2) as exppool,
            tc.tile_pool(name="kgpool", bufs=1) as kgpool,
            tc.tile_pool(name="vgpool", bufs=2) as vgpool,
            tc.tile_pool(name="rpool", bufs=4) as rpool,
            tc.tile_pool(name="opool", bufs=3) as opool,
            tc.tile_pool(name="ps_a", bufs=2, space="PSUM") as ps_a,
            tc.tile_pool(name="ps_s", bufs=3, space="PSUM") as ps_s,
            tc.tile_pool(name="ps_o", bufs=3, space="PSUM") as ps_o,
        ):
            # ---- persistent SBUF tiles -------------------------------------
            hsT_sb = consts.tile([128, 8 * Tq], F32R)
            wq_sb = consts.tile([128, 8 * 256], F32R)
            wk_sb = consts.tile([128, HPC * 8 * D], F32R)
            wv_sb = consts.tile([128, HPC * 8 * D], F32R)
            wo_sb = consts.tile([128, 2 * E], F32R)
            ident = consts.tile([128, 128], F32)
            qt_sb = [consts.tile([D, Tq], F32R, name=f"qt{h}") for h in range(HPC)]
            outT_sb = [consts.tile([128, Tq], F32R, name=f"outT{dd}") for dd in range(2)]

            make_identity(nc, ident)
            ones_f = consts.tile([1, 512], F32)
            nc.vector.memset(ones_f[:], 1.0)
            ones_sb = consts.tile([1, 512], F32R)
            nc.vector.tensor_copy(ones_sb[:], ones_f[:])
            onescol_f = consts.tile([128, HPC, 1], F32)
            nc.vector.memset(onescol_f[:], 1.0)

            # ---- input DMAs ------------------------------------------------
            for e in range(8):
                dma(out=wq_sb[:, e * 256:(e + 1) * 256], in_=wqT[e])
            for e in range(8):
                dma(out=hsT_sb[:, e * Tq:(e + 1) * Tq], in_=hsT[e])
            for h in range(HPC):
                for e in range(8):
                    dma(
                        out=wk_sb[:, (h * 8 + e) * D:(h * 8 + e + 1) * D],
                        in_=wkT[h, e])
                    dma(
                        out=wv_sb[:, (h * 8 + e) * D:(h * 8 + e + 1) * D],
                        in_=wvT[h, e])
            for dd in range(2):
                dma(out=wo_sb[:, dd * E:(dd + 1) * E], in_=woT[dd])

            bq_tiles = [consts.tile([D, 1], F32, name=f"bq{h}") for h in range(HPC)]
            for h in range(HPC):
                dma(out=bq_tiles[h][:], in_=bqh[h])

            # ---- phase 1: QT projection ------------------------------------
            # psum [128, 512] holds a head pair (rows 0-63 head 2p, 64-127 head 2p+1)
            for pair in range(2):
                for tqt in range(2):
                    ps_qt = ps_s.tile([128, 512], F32, tag="ps_s")
                    for e in range(8):
                        nc.tensor.matmul(
                            ps_qt[:],
                            r(wq_sb[:, e * 256 + pair * 128: e * 256 + (pair + 1) * 128]),
                            r(hsT_sb[:, e * Tq + tqt * 512: e * Tq + tqt * 512 + 512]),
                            start=(e == 0), stop=(e == 7))
                    for sub in range(2):
                        h = 2 * pair + sub
                        nc.scalar.activation(
                            qt_sb[h][:, tqt * 512: tqt * 512 + 512],
                            ps_qt[sub * 64:(sub + 1) * 64, :],
                            mybir.ActivationFunctionType.Identity,
                            bias=bq_tiles[h][:])

            # ---- phase 2: per-head K/V proj + attention --------------------
            for h in range(HPC):
                kvg_sb = kvpool.tile([128, 8 * M], F32R, tag="kvg")
                for e in range(8):
                    dma(
                        out=kvg_sb[:, e * M:(e + 1) * M], in_=kvgT[h, e])

                # K^T_g [64, 512]
                kg_sb = kgpool.tile([D, M], F32R, tag="kg", bufs=2)
                ps_kg = ps_a.tile([D, M], F32, tag="ps_a")
                for e in range(8):
                    nc.tensor.matmul(
                        ps_kg[:],
                        r(wk_sb[:, (h * 8 + e) * D:(h * 8 + e + 1) * D]),
                        r(kvg_sb[:, e * M:(e + 1) * M]),
                        start=(e == 0), stop=(e == 7))
                nc.vector.tensor_copy(kg_sb[:], ps_kg[:])

                # V^T_g [64, 512] -> transpose into Vg' [4][128, 65]
                vgT_sb = vgpool.tile([D, M], F32, tag="vgT")
                ps_vg = ps_a.tile([D, M], F32, tag="ps_a")
                for e in range(8):
                    nc.tensor.matmul(
                        ps_vg[:],
                        r(wv_sb[:, (h * 8 + e) * D:(h * 8 + e + 1) * D]),
                        r(kvg_sb[:, e * M:(e + 1) * M]),
                        start=(e == 0), stop=(e == 7))
                nc.vector.tensor_copy(vgT_sb[:], ps_vg[:])

                vgp_sb = vgpool.tile([128, 4, D + 1], F32R, tag="vgp")
                nc.vector.tensor_copy(vgp_sb[:, :, D:D + 1], onescol_f[:])
                for mc in range(4):
                    ps_vt = ps_a.tile([128, D], F32, tag="ps_a")
                    nc.tensor.transpose(
                        ps_vt[:], vgT_sb[:, mc * 128:(mc + 1) * 128],
                        ident[0:D, 0:D])
                    nc.vector.tensor_copy(vgp_sb[:, mc, 0:D], ps_vt[:])

                # scoresT -> exp -> attnT @ Vg' -> normalize
                expT_sb = exppool.tile([128, 4, Tq], F32R, tag="expT")
                for tqt in range(2):
                    for mc in range(4):
                        ps_sc = ps_s.tile([128, 512], F32, tag="ps_s")
                        nc.tensor.matmul(
                            ps_sc[:],
                            r(kg_sb[:, mc * 128:(mc + 1) * 128]),
                            r(qt_sb[h][:, tqt * 512: tqt * 512 + 512]),
                            start=True, stop=True)
                        nc.scalar.activation(
                            expT_sb[:, mc, tqt * 512: tqt * 512 + 512],
                            ps_sc[:],
                            mybir.ActivationFunctionType.Exp)

                    ps_ov = ps_o.tile([128, 512], F32, tag="ps_o")
                    for mc in range(4):
                        nc.tensor.matmul(
                            ps_ov[0:D + 1, :],
                            r(vgp_sb[:, mc, :]),
                            r(expT_sb[:, mc, tqt * 512: tqt * 512 + 512]),
                            start=(mc == 0), stop=(mc == 3))
                    rinv = rpool.tile([1, 512], F32R, tag="rinv")
                    with nc.allow_low_precision(reason="fp32r feed for PE bcast"):
                        nc.vector.reciprocal(rinv[:], ps_ov[D:D + 1, :])
                    ps_rb = ps_a.tile([D, 512], F32, tag="ps_a")
                    nc.tensor.matmul(ps_rb[:], ones_sb[:, 0:D], rinv[:],
                                     start=True, stop=True)
                    rinv_b = rpool.tile([D, 512], F32, tag="rinv_b")
                    nc.vector.tensor_copy(rinv_b[:], ps_rb[:])
                    nc.vector.tensor_tensor(
                        outT_sb[h // 2][(h % 2) * D:(h % 2 + 1) * D,
                                        tqt * 512: tqt * 512 + 512],
                        ps_ov[0:D, :],
                        rinv_b[:],
                        op=mybir.AluOpType.mult)

            # ---- phase 3: out-projection (row-parallel partial) ------------
            for t8 in range(8):
                for eot in range(2):
                    ps_po = ps_o.tile([128, 512], F32, tag="ps_o")
                    for dd in range(2):
                        nc.tensor.matmul(
                            ps_po[:],
                            r(outT_sb[dd][:, t8 * 128:(t8 + 1) * 128]),
                            r(wo_sb[:, dd * E + eot * 512: dd * E + eot * 512 + 512]),
                            start=(dd == 0), stop=(dd == 1))
                    po_sb = opool.tile([128, 512], F32, tag="po")
                    nc.vector.tensor_copy(po_sb[:], ps_po[:])
                    dma(
                        out=out[t8][:, eot * 512: eot * 512 + 512], in_=po_sb[:])

    nc.compile()
    return nc


_NC = None


def _get_nc():
    global _NC
    if _NC is None:
        _NC = build_program()
    return _NC


def round_fp32r(a):
    """Round-to-nearest-even to fp32r: 11-bit mantissa, low 12 bits zero."""
    u = np.ascontiguousarray(a, np.float32).view(np.uint32)
    u = u + 0x7FF + ((u >> 12) & 1)
    u &= np.uint32(0xFFFFF000)
    return u.view(np.float32)


def shard_inputs(hidden_states, key_value_states, Wq, bq, Wk, bk, Wv, bv, Wo, bo,
                 stride):
    stride = int(stride)
    assert stride == STRIDE
    scale = float(D) ** -0.5
    in_maps = []
    for c in range(NCORES):
        b, g = divmod(c, 4)
        h0 = g * HPC  # first global head of this core
        r0, r1 = h0 * D, (h0 + HPC) * D
        hsT_c = np.ascontiguousarray(hidden_states[b].T).reshape(8, 128, Tq)
        kvgT_c = np.empty((HPC, 8, 128, M), np.float32)
        for hl in range(HPC):
            rows = key_value_states[b, (h0 + hl)::STRIDE, :]  # [M, E]
            kvgT_c[hl] = np.ascontiguousarray(rows.T).reshape(8, 128, M)
        wqT_c = np.ascontiguousarray((Wq[r0:r1, :] * scale).T).reshape(8, 128, 256)
        bq_c = (bq[r0:r1] * scale).astype(np.float32).reshape(HPC, D, 1)
        wkT_c = np.ascontiguousarray(
            Wk[r0:r1, :].reshape(HPC, D, E).transpose(0, 2, 1)).reshape(
                HPC, 8, 128, D)
        wvT_c = np.ascontiguousarray(
            Wv[r0:r1, :].reshape(HPC, D, E).transpose(0, 2, 1)).reshape(
                HPC, 8, 128, D)
        woT_c = np.ascontiguousarray(Wo[:, r0:r1].T).reshape(2, 128, E)
        in_maps.append({
            "hsT": round_fp32r(hsT_c),
            "kvgT": round_fp32r(kvgT_c),
            "wqT": round_fp32r(wqT_c),
            "bqh": bq_c,
            "wkT": round_fp32r(wkT_c),
            "wvT": round_fp32r(wvT_c),
            "woT": round_fp32r(woT_c),
        })
    return in_maps


def combine_outputs(results, Wv, bv, Wo, bo):
    final_bias = (bv @ Wo.T + bo).astype(np.float32)  # [E]
    out = np.zeros((B, Tq, E), np.float32)
    for c in range(NCORES):
        b = c // 4
        out[b] += results[c]["out"].reshape(Tq, E)
    out += final_bias[None, None, :]
    return out


def kernel(hidden_states, key_value_states, Wq, bq, Wk, bk, Wv, bv, Wo, bo,
           stride, _trace=False, _trace_kwargs=None):
    from concourse.bass_utils import run_bass_kernel_spmd

    args = [np.asarray(x, np.float32) for x in
            (hidden_states, key_value_states, Wq, bq, Wk, bk, Wv, bv, Wo, bo)]
    (hidden_states, key_value_states, Wq, bq, Wk, bk, Wv, bv, Wo, bo) = args
    in_maps = shard_inputs(hidden_states, key_value_states, Wq, bq, Wk, bk,
                           Wv, bv, Wo, bo, stride)
    nc = _get_nc()
    res = run_bass_kernel_spmd(
        nc, in_maps, list(range(NCORES)),
        trace=_trace, **(_trace_kwargs or {}))
    out = combine_outputs(res.results, Wv, bv, Wo, bo)
    kernel.last_run = res
    return out
